# revision 27
# baseline (speedup 1.0000x reference)
"""Trainium2 Bass kernel for nn_DP_Attention (B=2, S=2048, D=1024, H=16, dh=64).

Key observation: the reference does a RAW reshape (B, LQ, D) -> (B, H, LQ, dh)
with no transpose.  Head h's (2048 x 64) q/k/v views are exactly the
(128-token x 1024-dim) row block [128h, 128h+128) of the projected activations,
reinterpreted.  Attention output rows for head h land back in the same token
block.  Hence: shard by (batch, token-block-of-512) across 8 cores; each core
computes 4 heads fully independently.  No collectives at all.

Per-core pipeline (all on one NeuronCore, Tile framework):
  1. qT/kT/vT = W @ X^T + b   (transposed projections, PE + ACT bias drain)
  2. per head:
     dist path:  S[q',k] = qh^T-chunk . khT  -> exp(S/32) on ACT with fused
                 row-sum accum -> DVE per-partition scale by 1/sum -> DMA out
     attn path:  S_T'[k',q'] (chunk-permuted k) -> exp -> PSUM-accumulated
                 attn_T = vh'_T . expS_T' -> normalize -> A-layout
  3. out^T = WO^T-contraction over A + bO -> DMA
Score matmuls run in float32r (4x faster than fp32 on the PE, ~1.5e-4 rel).
"""
import sys

for _p in ("/opt/trn_rl_repo", "/root/.axon_site/_ro/trn_rl_repo"):
    if _p not in sys.path:
        sys.path.append(_p)

import numpy as np
import concourse.bacc as bacc
import concourse.tile as tile
from concourse import mybir
from concourse.bass_utils import run_bass_kernel_spmd
from concourse.masks import make_identity

F32 = mybir.dt.float32
MM_DT = mybir.dt.float32r  # matmul operand dtype (float32r = fast PE path)
AF = mybir.ActivationFunctionType

B, S, D, H, DH = 2, 2048, 1024, 16, 64
N_CORES = 8
TPC = S * B // N_CORES          # tokens per core = 512
HPC = 4                          # heads per core
SCALE = 1.0 / 32.0               # 1 / (dh/2)


def build_nc():
    nc = bacc.Bacc("TRN2", target_bir_lowering=False, debug=False)

    # ---- DRAM I/O (per-core views; same program on all 8 cores) ----
    qt = nc.dram_tensor("qt", [D, TPC], MM_DT, kind="ExternalInput")
    kt = nc.dram_tensor("kt", [D, TPC], MM_DT, kind="ExternalInput")
    vt = nc.dram_tensor("vt", [D, TPC], MM_DT, kind="ExternalInput")
    wq = nc.dram_tensor("wq", [D, D], MM_DT, kind="ExternalInput")
    wk = nc.dram_tensor("wk", [D, D], MM_DT, kind="ExternalInput")
    wv = nc.dram_tensor("wv", [D, D], MM_DT, kind="ExternalInput")
    wo = nc.dram_tensor("wo", [D, D], MM_DT, kind="ExternalInput")
    bq = nc.dram_tensor("bq", [128, 8], F32, kind="ExternalInput")
    bk = nc.dram_tensor("bk", [128, 8], F32, kind="ExternalInput")
    bv = nc.dram_tensor("bv", [128, 8], F32, kind="ExternalInput")
    bo = nc.dram_tensor("bo", [128, 8], F32, kind="ExternalInput")
    out_t = nc.dram_tensor("out_t", [D, TPC], F32, kind="ExternalOutput")
    dist = nc.dram_tensor("dist", [HPC, S, S], F32, kind="ExternalOutput")

    dist_r = dist.rearrange("h (t c) k -> h c t k", c=16)   # [4,16,128,2048]
    out_r = out_t.rearrange("(j p) t -> j p t", p=128)      # [8,128,512]

    with tile.TileContext(nc) as tc:
        import contextlib
        with contextlib.ExitStack() as ctx:
            wt_pool = ctx.enter_context(tc.tile_pool(name="wt", bufs=3))
            xin_pool = ctx.enter_context(tc.tile_pool(name="xin", bufs=2))
            qkvt_pool = ctx.enter_context(tc.tile_pool(name="qkvt", bufs=1))
            const_pool = ctx.enter_context(tc.tile_pool(name="const", bufs=1))
            head_pool = ctx.enter_context(tc.tile_pool(name="head", bufs=2))
            vh_pool = ctx.enter_context(tc.tile_pool(name="vh", bufs=2))
            distu_pool = ctx.enter_context(tc.tile_pool(name="distu", bufs=2))
            expa_pool = ctx.enter_context(tc.tile_pool(name="expa", bufs=3))
            a_pool = ctx.enter_context(tc.tile_pool(name="apool", bufs=1))
            small_pool = ctx.enter_context(tc.tile_pool(name="small", bufs=2))
            outt_pool = ctx.enter_context(tc.tile_pool(name="outt", bufs=2))
            ps1 = ctx.enter_context(tc.tile_pool(name="ps1", bufs=1, space="PSUM"))
            ps2 = ctx.enter_context(tc.tile_pool(name="ps2", bufs=2, space="PSUM"))
            psA = ctx.enter_context(tc.tile_pool(name="psA", bufs=1, space="PSUM"))

            # ---- constants ----
            id2 = const_pool.tile([128, 64], F32)      # identity in both halves
            make_identity(nc, id2[0:64, :])
            make_identity(nc, id2[64:128, :], nomemset=False)
            id128 = const_pool.tile([128, 128], F32)
            make_identity(nc, id128)
            # rowsel[p, c*128+m] = 1 if p == c else 0   (p<16, c<16, m<128)
            rowsel = const_pool.tile([16, 2048], F32)
            nc.gpsimd.memset(rowsel, 0.0)
            nc.gpsimd.affine_select(
                out=rowsel, in_=rowsel,
                compare_op=mybir.AluOpType.not_equal, fill=1.0, base=0,
                pattern=[[-1, 16], [0, 128]], channel_multiplier=1)
            bias_q = const_pool.tile([128, 8], F32)
            bias_k = const_pool.tile([128, 8], F32)
            bias_v = const_pool.tile([128, 8], F32)
            bias_o = const_pool.tile([128, 8], F32)
            nc.sync.dma_start(out=bias_q, in_=bq[:, :])
            nc.sync.dma_start(out=bias_k, in_=bk[:, :])
            nc.sync.dma_start(out=bias_v, in_=bv[:, :])
            nc.sync.dma_start(out=bias_o, in_=bo[:, :])

            # ---- projections: xT_j[j][p, t] = (W @ X^T + b)[j*128+p, t] ----
            qT_j = [qkvt_pool.tile([128, 512], MM_DT, tag=f"qT{j}", name=f"qT{j}") for j in range(8)]
            kT_j = [qkvt_pool.tile([128, 512], MM_DT, tag=f"kT{j}", name=f"kT{j}") for j in range(8)]
            vT_j = [qkvt_pool.tile([128, 512], F32, tag=f"vT{j}", name=f"vT{j}") for j in range(8)]

            def proj(x_dram, w_dram, bias_t, out_js):
                x_s = xin_pool.tile([128, 8, 512], MM_DT, tag="xin", name="x_s")
                nc.sync.dma_start(
                    out=x_s, in_=x_dram.rearrange("(i p) t -> p i t", p=128))
                w_r = w_dram.rearrange("(i p) dout -> p i dout", p=128)
                for j in range(8):
                    w_j = wt_pool.tile([128, 8, 128], MM_DT, tag="wt", name=f"w_{j}")
                    nc.sync.dma_start(out=w_j, in_=w_r[:, :, j * 128:(j + 1) * 128])
                    psum = ps2.tile([128, 512], F32, tag="ps2", name=f"psp_{j}")
                    for i in range(8):
                        nc.tensor.matmul(
                            psum[:, :], w_j[:, i, :], x_s[:, i, :],
                            start=(i == 0), stop=(i == 7))
                    nc.vector.tensor_scalar_add(
                        out_js[j][:, :], psum[:, :], bias_t[:, j:j + 1])

            heads = [dict() for _ in range(HPC)]
            A = a_pool.tile([128, 4096], MM_DT)
            A_hx = A.rearrange("p (hh x) -> p hh x", hh=4)

            def stage_kh(hh):
                tb = 128 * hh
                khT2 = head_pool.tile([128, 2048], MM_DT, tag="khT2",
                                      name=f"khT2_{hh}")
                khT2_r = khT2.rearrange("p (t c) -> p c t", c=16)
                for c in range(16):
                    src_k = kT_j[c // 2][(c % 2) * 64:(c % 2) * 64 + 64,
                                         tb:tb + 128]
                    for half in range(2):
                        nc.vector.tensor_copy(
                            khT2_r[half * 64:half * 64 + 64, c, :], src_k)
                heads[hh]["khT2"] = khT2
                heads[hh]["sums_col"] = small_pool.tile(
                    [128, 16], F32, tag="sums", name=f"sums_{hh}")
                heads[hh]["recip_col"] = small_pool.tile(
                    [128, 16], F32, tag="recip", name=f"recip_{hh}")

            def stage_qh(hh):
                tb = 128 * hh
                qhT2 = head_pool.tile([128, 2048], MM_DT, tag="qhT2",
                                      name=f"qhT2_{hh}")
                for c in range(16):
                    src_q = qT_j[c // 2][(c % 2) * 64:(c % 2) * 64 + 64,
                                         tb:tb + 128]
                    for half in range(2):
                        nc.vector.tensor_copy(
                            qhT2[half * 64:half * 64 + 64, c * 128:(c + 1) * 128],
                            src_q)
                heads[hh]["qhT2"] = qhT2

            def stage_vh(hh):
                tb = 128 * hh
                vh_s = vh_pool.tile([128, 1024], MM_DT, tag="vh", name=f"vh_{hh}")
                for ck in range(16):
                    pst = ps2.tile([128, 64], F32, tag="ps2", name=f"pst_{hh}_{ck}")
                    nc.tensor.transpose(
                        pst[:, :],
                        vT_j[ck // 2][(ck % 2) * 64:(ck % 2) * 64 + 64,
                                      tb:tb + 128],
                        id2[(ck % 2) * 64:(ck % 2) * 64 + 64, :])
                    nc.vector.tensor_copy(vh_s[:, ck * 64:(ck + 1) * 64], pst[:, :])
                heads[hh]["vh"] = vh_s


            def dist_cq(hh, cq):
                tb = 128 * hh
                khT2 = heads[hh]["khT2"]
                sums_col = heads[hh]["sums_col"]
                recip_col = heads[hh]["recip_col"]
                lhs_q = qT_j[cq // 2][(cq % 2) * 64:(cq % 2) * 64 + 64,
                                      tb:tb + 128]
                distU = distu_pool.tile([128, 2048], F32, tag="distU",
                                        name=f"distU_{hh}_{cq}")
                part = small_pool.tile([128, 2], F32, tag="part",
                                       name=f"part_{hh}_{cq}")
                for x in range(2):
                    psum = ps1.tile([128, 1024], F32, tag="ps1",
                                    name=f"psd_{hh}_{cq}_{x}")
                    for y in range(2):
                        ko = x * 1024 + y * 512
                        nc.tensor.matmul(
                            psum[:, y * 512:(y + 1) * 512], lhs_q,
                            khT2[(cq % 2) * 64:(cq % 2) * 64 + 64, ko:ko + 512],
                            start=True, stop=True)
                    nc.scalar.activation(
                        distU[:, x * 1024:(x + 1) * 1024], psum[:, :],
                        AF.Exp, scale=SCALE, accum_out=part[:, x:x + 1])
                nc.vector.tensor_add(
                    sums_col[:, cq:cq + 1], part[:, 0:1], part[:, 1:2])
                nc.vector.reciprocal(
                    recip_col[:, cq:cq + 1], sums_col[:, cq:cq + 1])
                for x in range(2):
                    nc.vector.tensor_scalar_mul(
                        distU[:, x * 1024:(x + 1) * 1024],
                        distU[:, x * 1024:(x + 1) * 1024],
                        recip_col[:, cq:cq + 1])
                nc.sync.dma_start(out=dist_r[hh, cq], in_=distU[:, :])

            def attn_open(hh, half):
                heads[hh][f"ps_at{half}"] = psA.tile(
                    [64, 1024], F32, tag="psA", name=f"psat_{hh}_{half}")

            def attn_unit_half(hh, half, ck):
                tb = 128 * hh
                qhT2 = heads[hh]["qhT2"]
                vh_s = heads[hh]["vh"]
                ps_at = heads[hh][f"ps_at{half}"]
                lhs_k = kT_j[ck // 2][(ck % 2) * 64:(ck % 2) * 64 + 64,
                                      tb:tb + 128]
                ps_st = ps2.tile([128, 1024], F32, tag="ps2",
                                 name=f"psst_{hh}_{half}_{ck}")
                for y in range(2):
                    qo = half * 1024 + y * 512
                    nc.tensor.matmul(
                        ps_st[:, y * 512:(y + 1) * 512], lhs_k,
                        qhT2[(ck % 2) * 64:(ck % 2) * 64 + 64, qo:qo + 512],
                        start=True, stop=True)
                expS = expa_pool.tile([128, 1024], MM_DT, tag="expS",
                                      name=f"expS_{hh}_{half}_{ck}")
                nc.scalar.activation(expS[:, :], ps_st[:, :],
                                     AF.Exp, scale=SCALE)
                for y in range(2):
                    nc.tensor.matmul(
                        ps_at[:, y * 512:(y + 1) * 512],
                        vh_s[:, ck * 64:(ck + 1) * 64],
                        expS[:, y * 512:(y + 1) * 512],
                        start=(ck == 0), stop=(ck == 15))

            def recip_bcast(hh):
                recip_col = heads[hh]["recip_col"]
                psT = ps2.tile([16, 128], F32, tag="ps2", name=f"psT_{hh}")
                nc.tensor.transpose(psT[:, :], recip_col[:, :], id128[:, :])
                recipT = small_pool.tile([16, 128], F32, tag="recipT",
                                         name=f"recipT_{hh}")
                nc.vector.tensor_copy(recipT[:, :], psT[:, :])
                recip_b = small_pool.tile([128, 2048], F32, tag="recipb",
                                          name=f"recipb_{hh}")
                for xh in range(2):
                    psb = ps2.tile([128, 1024], F32, tag="ps2",
                                   name=f"psb_{hh}_{xh}")
                    for cq in range(8):
                        c = xh * 8 + cq
                        nc.tensor.matmul(
                            psb[:, cq * 128:(cq + 1) * 128],
                            rowsel[:, c * 128:(c + 1) * 128], recipT[:, :],
                            start=True, stop=True)
                    nc.vector.tensor_copy(
                        recip_b[:, xh * 1024:(xh + 1) * 1024], psb[:, :])
                heads[hh]["recip_b"] = recip_b

            def a_writes_half(hh, half):
                ps_at = heads[hh][f"ps_at{half}"]
                recip_b = heads[hh]["recip_b"]
                for cc in range(8):
                    c = half * 8 + cc
                    nc.vector.tensor_mul(
                        A[(c % 2) * 64:(c % 2) * 64 + 64,
                          hh * 1024 + (c // 2) * 128:hh * 1024 + (c // 2) * 128 + 128],
                        ps_at[:, cc * 128:(cc + 1) * 128],
                        recip_b[0:64, c * 128:(c + 1) * 128])

            # ---- emission order: software pipeline over heads ----
            proj(kt, wk, bias_k, kT_j)
            stage_kh(0)
            proj(qt, wq, bias_q, qT_j)
            stage_qh(0)
            for cq in range(6):
                dist_cq(0, cq)
            proj(vt, wv, bias_v, vT_j)
            stage_vh(0)
            attn_open(0, 0)
            for u in range(16):
                if u + 6 < 16:
                    dist_cq(0, u + 6)
                attn_unit_half(0, 0, u)
            recip_bcast(0)
            for hh in range(1, HPC):
                stage_kh(hh)
                stage_qh(hh)
                attn_open(hh - 1, 1)
                for u in range(16):
                    attn_unit_half(hh - 1, 1, u)
                    dist_cq(hh, u)
                a_writes_half(hh - 1, 0)
                a_writes_half(hh - 1, 1)
                stage_vh(hh)
                attn_open(hh, 0)
                for u in range(16):
                    attn_unit_half(hh, 0, u)
                recip_bcast(hh)
            attn_open(3, 1)
            for u in range(16):
                attn_unit_half(3, 1, u)
            a_writes_half(3, 0)
            a_writes_half(3, 1)

            # ---- out projection ----
            wo_r = wo.rearrange("(i p) dm -> p i dm", p=128)
            for j in range(8):
                w_j = wt_pool.tile([128, 8, 128], MM_DT, tag="wt", name=f"wo_{j}")
                nc.sync.dma_start(out=w_j, in_=wo_r[:, :, j * 128:(j + 1) * 128])
                psum = ps2.tile([128, 512], F32, tag="ps2", name=f"pso_{j}")
                for i in range(8):
                    nc.tensor.matmul(
                        psum[:, :], w_j[:, i, :],
                        A_hx[:, :, i * 128:(i + 1) * 128],
                        start=(i == 0), stop=(i == 7))
                o_t = outt_pool.tile([128, 512], F32, tag="outt", name=f"ot_{j}")
                nc.vector.tensor_scalar_add(o_t[:, :], psum[:, :],
                                            bias_o[:, j:j + 1])
                nc.sync.dma_start(out=out_r[j], in_=o_t[:, :])

    nc.compile()
    return nc


_NC = None


def _get_nc():
    global _NC
    if _NC is None:
        _NC = build_nc()
    return _NC


def kernel(Q, K, V, WQ, bQ, WK, bK, WV, bV, WO, bO, _trace=False, _trace_kwargs=None):
    nc = _get_nc()
    f32 = np.float32
    wq_t = np.ascontiguousarray(WQ.T.astype(f32))
    wk_t = np.ascontiguousarray(WK.T.astype(f32))
    wv_t = np.ascontiguousarray(WV.T.astype(f32))
    wo_t = np.ascontiguousarray(WO.T.astype(f32))
    b_r = lambda b: np.ascontiguousarray(np.asarray(b, f32).reshape(8, 128).T)
    bq_r, bk_r, bv_r, bo_r = b_r(bQ), b_r(bK), b_r(bV), b_r(bO)

    QT = [np.asarray(Q[b], f32).T for b in range(B)]
    KT = [np.asarray(K[b], f32).T for b in range(B)]
    VT = [np.asarray(V[b], f32).T for b in range(B)]

    in_maps = []
    for c in range(N_CORES):
        b, g = divmod(c, 4)
        sl = slice(g * TPC, (g + 1) * TPC)
        in_maps.append({
            "qt": np.ascontiguousarray(QT[b][:, sl]),
            "kt": np.ascontiguousarray(KT[b][:, sl]),
            "vt": np.ascontiguousarray(VT[b][:, sl]),
            "wq": wq_t, "wk": wk_t, "wv": wv_t, "wo": wo_t,
            "bq": bq_r, "bk": bk_r, "bv": bv_r, "bo": bo_r,
        })

    kwargs = {}
    if _trace:
        kwargs["trace"] = True
        if _trace_kwargs:
            kwargs.update(_trace_kwargs)
    res = run_bass_kernel_spmd(nc, in_maps, core_ids=list(range(N_CORES)), **kwargs)

    out = np.empty((B, S, D), f32)
    dist_full = np.empty((B, H, S, S), f32)
    for c in range(N_CORES):
        b, g = divmod(c, 4)
        r = res.results[c]
        out[b, g * TPC:(g + 1) * TPC, :] = r["out_t"].T
        dist_full[b, g * HPC:(g + 1) * HPC] = r["dist"]
    kernel._last_results = res
    return out, dist_full
